# revision 1
# baseline (speedup 1.0000x reference)
"""Trainium2 Bass kernel for nn_CGNN_83605833384509.

Banded-DAG CGNN: gen[:, n] = MLP_n(gen[:, n-4:n] masked, noise[:, n]),
n = 0..63 sequential, B = 262144 batch.

Strategy: data-parallel over 8 cores (B/8 = 32768 each). Per core, a
node-staggered software pipeline ("superwaves"): at superwave s node n
processes chunk c = s - n (chunks of W=512 columns). Generated values
live in a windowed, partition-replicated SBUF ring tensor X so every
matmul reads/writes 32-aligned partition windows (walrus requirement).
Per node: z = W1g.gen_parents + W1n.noise + b1 via 5 accumulating
32x32-tile matmuls (3 nodes packed per matmul, float32r), relu via
ACT/DVE psum->SBUF evacuation, y = W2.h + b2 via embedded-column
matmuls, all 64 active nodes' y written back to X in one 128-lane op.
Noise streams in / gen streams out via diagonal-in-DRAM DMA patterns.
"""

import numpy as np

# ---------------------------------------------------------------- constants
NN = 64          # nodes
KP = 4           # max parents
NH = 10          # hidden width
W = 512          # chunk width (psum bank = 512 fp32)
C = 64           # chunks per core: B_shard = C*W = 32768
B_SHARD = C * W
N_CORES = 8
B_FULL = B_SHARD * N_CORES
NSTREAM = 4               # independent chunk-range streams (pipeline overlap)
CS = C // NSTREAM         # chunks per stream
NSW = CS + NN - 1         # superwaves per stream = 95
XRING = 32                # gen ring slots total (16 per stream)
XR_S = XRING // NSTREAM
NRING = 16                # noise ring slots total (8 per stream)
NR_S = NRING // NSTREAM
NLAG = 2                  # noise refresh lead (superwaves), < NR_S
HQ = 2                    # Hbuf ring depth per stream
NZB = 6                   # z psum banks

# Windows: quadrant q holds gen rows for nodes [wlo, whi] at partition
# 32*q + (m - wlo).  Every trio's parents+self fit in its own window.
WIN = [(0, 14), (8, 29), (24, 45), (40, 63)]
# trio tau = nodes 3t..3t+2 (trio 21 = node 63 only)
NTRIO = 22


def trio_nodes(tau):
    return [n for n in range(3 * tau, min(3 * tau + 3, NN))]


def trio_win(tau):
    n0 = 3 * tau
    if n0 <= 12:
        return 0
    if n0 <= 27:
        return 1
    if n0 <= 42:
        return 2
    return 3


def win_rows(q):
    lo, hi = WIN[q]
    return hi - lo + 1


def pos_in_win(m, q):
    """partition row of gen-node m inside window q (must be present)."""
    lo, hi = WIN[q]
    assert lo <= m <= hi, (m, q)
    return 32 * q + (m - lo)


def windows_of(m):
    return [q for q in range(4) if WIN[q][0] <= m <= WIN[q][1]]


def primary_win(m):
    """primary window: q0: 0-14, q1: 15-29, q2: 30-45, q3: 46-63."""
    if m <= 14:
        return 0
    if m <= 29:
        return 1
    if m <= 45:
        return 2
    return 3


# z-psum placement: trio tau -> (zq, zb): quadrant zq = tau % 4, bank
# zb = tau // 4 (6 banks).  z rows = 32*zq .. 32*zq+29 (3 nodes x 10).
def trio_zq(tau):
    return tau % 4


def trio_zb(tau):
    return tau // 4


def active_range(s):
    return max(0, s - CS + 1), min(NN - 1, s)


def trio_active(tau, s):
    lo, hi = active_range(s)
    ns = trio_nodes(tau)
    return ns[0] <= hi and ns[-1] >= lo


# ------------------------------------------------------------- weight packing
def w1_row_for_parent(n, j):
    """W1 slot row holding the weight of parent m = n - j for node n."""
    if n >= KP:
        return KP - j
    return n - j  # left-aligned parents for n < 4


def pack_weights(W1, b1, W2, b2):
    """Build packed host arrays for the kernel.

    Returns dict with:
      wph  [128, NTRIO*5*30]  phase lhsT blocks (j=0 noise+bias, 1..4 parents)
      wl2  [128, L2COLS]      L2 embedded lhsT segments
      b2c  [128, 1]           y-evac bias (b2 at every window position)
      segs list of (tau, oq, row_a, mseg, col_off)  L2 segment table
      phase_nz [NTRIO][5]     whether the phase block is nonzero
    """
    W1 = np.asarray(W1, np.float32)
    b1 = np.asarray(b1, np.float32)
    W2 = np.asarray(W2, np.float32)
    b2 = np.asarray(b2, np.float32)

    wph = np.zeros((128, NTRIO * 5 * 30), np.float32)
    phase_nz = np.zeros((NTRIO, 5), bool)
    for tau in range(NTRIO):
        q = trio_win(tau)
        for j in range(5):
            off = (tau * 5 + j) * 30
            blk = wph[:, off:off + 30]
            for i, n in enumerate(trio_nodes(tau)):
                if j == 0:
                    # noise weights at node's own row + bias on ones-row 31
                    blk[pos_in_win(n, q), 10 * i:10 * i + 10] = W1[n, KP]
                    blk[32 * q + 31, 10 * i:10 * i + 10] = b1[n]
                    phase_nz[tau, j] = True
                else:
                    m = n - j
                    if m < 0:
                        continue
                    blk[pos_in_win(m, q), 10 * i:10 * i + 10] = \
                        W1[n, w1_row_for_parent(n, j)]
                    phase_nz[tau, j] = True

    # L2: one full-array (128 x 128) lhsT per z-bank: contracts the bank's
    # whole Hbuf column (its 4 trios), writes y at every window position of
    # its nodes (zero columns elsewhere); banks accumulate into y psum.
    segs = list(range(NZB))
    wl2 = np.zeros((128, NZB * 128), np.float32)
    for zb in range(NZB):
        blk = wl2[:, zb * 128:(zb + 1) * 128]
        for t in range(zb * 4, min(zb * 4 + 4, NTRIO)):
            zq = trio_zq(t)
            for i, n in enumerate(trio_nodes(t)):
                for oq in windows_of(n):
                    blk[32 * zq + 10 * i:32 * zq + 10 * i + 10,
                        pos_in_win(n, oq)] = W2[n]
    l2cols = NZB * 128

    b2c = np.zeros((128, 1), np.float32)
    for m in range(NN):
        for q in windows_of(m):
            b2c[pos_in_win(m, q), 0] = b2[m]

    return dict(wph=wph, wl2=wl2, b2c=b2c, segs=segs, phase_nz=phase_nz,
                l2cols=l2cols)


# ------------------------------------------------------------- schedule
def xn_dma_jobs(sp):
    """Noise-refresh DMA jobs for superwave sp: list of
    (quad, row_a, nrows, n_lo, ring_slot, c_lo).  SBUF rows row_a.. get
    noise rows n_lo.. at chunk offsets c = sp - n (linear in n)."""
    lo, hi = active_range(sp)
    jobs = []
    # group active nodes by their trio window (contiguous node ranges)
    by_q = {}
    for n in range(lo, hi + 1):
        q = trio_win(n // 3)
        by_q.setdefault(q, []).append(n)
    for q, ns in sorted(by_q.items()):
        n_lo, n_hi = ns[0], ns[-1]
        assert ns == list(range(n_lo, n_hi + 1))
        row_a = pos_in_win(n_lo, q)
        jobs.append((q, row_a, n_hi - n_lo + 1, n_lo, sp % NR_S, sp - n_lo))
    return jobs


def out_dma_jobs(sg):
    """Gen DMA-out jobs for slot written at superwave sg: list of
    (quad, row_a, nrows, m_lo, ring_slot, c_lo)."""
    lo, hi = active_range(sg)
    jobs = []
    bounds = [(0, 14), (15, 29), (30, 45), (46, 63)]
    for q, (plo, phi) in enumerate(bounds):
        m_lo, m_hi = max(lo, plo), min(hi, phi)
        if m_lo > m_hi:
            continue
        row_a = pos_in_win(m_lo, q)
        jobs.append((q, row_a, m_hi - m_lo + 1, m_lo, sg % XR_S, sg - m_lo))
    return jobs


# ------------------------------------------------------------- numpy emulator
def emulate_core(noiseT, packed, w=W, c=C):
    """Pure-numpy emulation of the exact kernel schedule (streamed)."""
    cs = c // NSTREAM
    nsw = cs + NN - 1
    wph, wl2, b2c = packed["wph"], packed["wl2"], packed["b2c"]
    segs, phase_nz = packed["segs"], packed["phase_nz"]

    X = np.zeros((128, XRING * w), np.float32)
    XN = np.zeros((128, NRING * w), np.float32)
    XN[[31, 63, 95, 127], :] = 1.0
    Hbuf = np.zeros((128, NSTREAM * HQ * NZB * w), np.float32)
    zps = np.zeros((NZB, 128, w), np.float32)
    yps = np.zeros((2, 128, w), np.float32)
    G = np.zeros((NN, c * w), np.float32)

    def xn_refresh(sg, sp):
        if sp >= nsw:
            return
        cb = sg * cs
        for (q, row_a, nrows, n_lo, rs, c_lo) in xn_dma_jobs(sp):
            for k in range(nrows):
                cc = c_lo - k
                if 0 <= cc < cs:
                    XN[row_a + k, (sg * NR_S + rs) * w:(sg * NR_S + rs + 1) * w] = \
                        noiseT[n_lo + k, (cb + cc) * w:(cb + cc + 1) * w]

    def dma_out(sg, so):
        cb = sg * cs
        for (q, row_a, nrows, m_lo, rs, c_lo) in out_dma_jobs(so):
            for k in range(nrows):
                cc = c_lo - k
                assert 0 <= cc < cs
                G[m_lo + k, (cb + cc) * w:(cb + cc + 1) * w] = \
                    X[row_a + k, (sg * XR_S + rs) * w:(sg * XR_S + rs + 1) * w]

    for sg in range(NSTREAM):
        for sp in range(min(NLAG, nsw)):
            xn_refresh(sg, sp)

    for t in range(nsw):
        for sg in range(NSTREAM):
            s = t
            xn_refresh(sg, s + NLAG)
            act_trios = [tt for tt in range(NTRIO) if trio_active(tt, s)]
            for tau in act_trios:
                q, zq = trio_win(tau), trio_zq(tau)
                zb = trio_zb(tau)
                js = [j for j in (0, 4, 3, 2, 1) if phase_nz[tau, j]]
                first = True
                for j in js:
                    off = (tau * 5 + j) * 30
                    if j == 0:
                        sl = sg * NR_S + (s % NR_S)
                        rhs = XN[32 * q:32 * q + 32, sl * w:(sl + 1) * w]
                        lhsT = wph[32 * q:32 * q + 32, off:off + 30]
                    else:
                        kw = win_rows(q)
                        sl = sg * XR_S + ((s - j) % XR_S)
                        rhs = X[32 * q:32 * q + kw, sl * w:(sl + 1) * w]
                        lhsT = wph[32 * q:32 * q + kw, off:off + 30]
                    contrib = lhsT.T @ rhs
                    if first:
                        zps[zb][32 * zq:32 * zq + 30, :] = contrib
                        first = False
                    else:
                        zps[zb][32 * zq:32 * zq + 30, :] += contrib
            act_banks = sorted({trio_zb(tt) for tt in act_trios})
            act_pairs = sorted({zb // 2 for zb in act_banks})
            for pb in act_pairs:
                for zb in (2 * pb, 2 * pb + 1):
                    hcol = (((sg * HQ) + (s % HQ)) * NZB + zb) * w
                    Hbuf[:, hcol:hcol + w] = np.maximum(zps[zb], 0.0)
            act_banks = [zb for pb in act_pairs for zb in (2*pb, 2*pb+1)
                         if zb < NZB]
            yp = yps[s % 2]
            acc = np.zeros((128, w), np.float32)
            for zb in act_banks:
                hcol = (((sg * HQ) + (s % HQ)) * NZB + zb) * w
                acc += wl2[:, zb * 128:(zb + 1) * 128].T @ Hbuf[:, hcol:hcol + w]
            yp[:, :] = acc
            sl = sg * XR_S + (s % XR_S)
            X[:, sl * w:(sl + 1) * w] = yp + b2c
            if s - 5 >= 0:
                dma_out(sg, s - 5)
    for so in range(max(0, nsw - 5), nsw):
        for sg in range(NSTREAM):
            dma_out(sg, so)
    return G


# ------------------------------------------------------------- bass kernel
def build_bass(w=W, c=C, l2cols=None, enable_asserts=False):
    import concourse.bass as bass
    import concourse.bacc as bacc
    import concourse.mybir as mybir
    import concourse.tile as tile

    nsw = c + NN - 1
    f32 = mybir.dt.float32
    bf16 = mybir.dt.bfloat16
    RELU = mybir.ActivationFunctionType.Relu

    nc = bacc.Bacc("TRN2", target_bir_lowering=False, debug=False,
                   enable_asserts=enable_asserts, num_devices=N_CORES)

    d_noise = nc.dram_tensor("noiseT", [NN, c * w], bf16, kind="ExternalInput").ap()
    d_wph = nc.dram_tensor("wph", [128, NTRIO * 5 * 30], bf16, kind="ExternalInput").ap()
    d_wl2 = nc.dram_tensor("wl2", [128, l2cols], bf16, kind="ExternalInput").ap()
    d_b2c = nc.dram_tensor("b2c", [128, 1], f32, kind="ExternalInput").ap()
    d_ones = nc.dram_tensor("ones", [4, NRING * w], bf16, kind="ExternalInput").ap()
    d_zero = nc.dram_tensor("zeros", [128, XRING * w], bf16, kind="ExternalInput").ap()
    d_gen = nc.dram_tensor("gen", [NN, c * w], bf16, kind="ExternalOutput").ap()

    # static tables shared with packing
    phase_nz = build_bass._phase_nz
    segs = build_bass._segs

    with tile.TileContext(nc) as tc:
        with tc.tile_pool(name="sb", bufs=1) as sb, \
             tc.tile_pool(name="ps", bufs=1, space="PSUM") as pp:
            cs = c // NSTREAM
            nsw = cs + NN - 1
            X = sb.tile([128, XRING * w], bf16)
            XN = sb.tile([128, NRING * w], bf16)
            Hbuf = sb.tile([128, NSTREAM * HQ * NZB * w], bf16)
            WPH = sb.tile([128, NTRIO * 5 * 30], bf16)
            WL2 = sb.tile([128, l2cols], bf16)
            B2C = sb.tile([128, 1], f32)
            zpt = [pp.tile([128, 2 * w], f32, name=f"zpt{i}")
                   for i in range(NZB // 2)]
            yps = [pp.tile([128, w], f32, name=f"yps{i}") for i in range(2)]

            nc.sync.dma_start(WPH[:], d_wph[:])
            nc.sync.dma_start(WL2[:], d_wl2[:])
            nc.sync.dma_start(B2C[:], d_b2c[:])
            nc.sync.dma_start(X[:], d_zero[:])
            nc.sync.dma_start(XN[:], d_zero[:, :NRING * w])
            for t in zpt:
                nc.vector.memset(t[:], 0.0)
            for t in yps:
                nc.vector.memset(t[:], 0.0)
            for qi in range(4):
                nc.sync.dma_start(XN[32 * qi + 31:32 * qi + 32, :],
                                  d_ones[qi:qi + 1, :])

            def xn_refresh(sg, sp):
                if sp >= nsw:
                    return
                cb = sg * cs
                for (q, row_a, nrows, n_lo, rs, c_lo) in xn_dma_jobs(sp):
                    k_ok = [k for k in range(nrows) if 0 <= c_lo - k < cs]
                    if not k_ok:
                        continue
                    k0, k1 = min(k_ok), max(k_ok)
                    off = (n_lo + k0) * c * w + (cb + c_lo - k0) * w
                    src_ap = bass.AP(d_noise.tensor, off,
                                     [[c * w - w, k1 - k0 + 1], [1, w]])
                    sl = sg * NR_S + rs
                    nc.sync.dma_start(
                        XN[row_a + k0:row_a + k1 + 1, sl * w:(sl + 1) * w],
                        src_ap)

            def dma_out(sg, so):
                cb = sg * cs
                for (q, row_a, nrows, m_lo, rs, c_lo) in out_dma_jobs(so):
                    off = m_lo * c * w + (cb + c_lo) * w
                    dst = bass.AP(d_gen.tensor, off,
                                  [[c * w - w, nrows], [1, w]])
                    sl = sg * XR_S + rs
                    nc.sync.dma_start(
                        dst, X[row_a:row_a + nrows, sl * w:(sl + 1) * w])

            for sg in range(NSTREAM):
                for sp in range(min(NLAG, nsw)):
                    xn_refresh(sg, sp)

            for t in range(nsw):
                for sg in range(NSTREAM):
                    s = t
                    xn_refresh(sg, s + NLAG)
                    act_trios = [tt for tt in range(NTRIO)
                                 if trio_active(tt, s)]
                    for tau in act_trios:
                        q, zq, zb = trio_win(tau), trio_zq(tau), trio_zb(tau)
                        js = [j for j in (0, 4, 3, 2, 1) if phase_nz[tau, j]]
                        for ji, j in enumerate(js):
                            off = (tau * 5 + j) * 30
                            if j == 0:
                                kw = 32
                                sl = sg * NR_S + (s % NR_S)
                                rhs = XN[32 * q:32 * q + 32,
                                         sl * w:(sl + 1) * w]
                            else:
                                kw = win_rows(q)
                                sl = sg * XR_S + ((s - j) % XR_S)
                                rhs = X[32 * q:32 * q + kw,
                                        sl * w:(sl + 1) * w]
                            lhsT = WPH[32 * q:32 * q + kw, off:off + 30]
                            nc.tensor.matmul(
                                zpt[zb // 2][32 * zq:32 * zq + 30,
                                             (zb % 2) * w:(zb % 2) * w + w],
                                lhsT, rhs,
                                start=(ji == 0), stop=(ji == len(js) - 1),
                                skip_group_check=True,
                                tile_position=(32 * q, 32 * zq))
                    act_banks0 = sorted({trio_zb(tt) for tt in act_trios})
                    act_pairs = sorted({zb // 2 for zb in act_banks0})
                    for bi, pb in enumerate(act_pairs):
                        hcol = (((sg * HQ) + (s % HQ)) * NZB + 2 * pb) * w
                        if bi % 2 == 0:
                            nc.scalar.activation(Hbuf[:, hcol:hcol + 2 * w],
                                                 zpt[pb][:], RELU)
                        else:
                            nc.vector.tensor_scalar_max(
                                Hbuf[:, hcol:hcol + 2 * w], zpt[pb][:], 0.0)
                    act_banks = [zb for pb in act_pairs
                                 for zb in (2 * pb, 2 * pb + 1)]
                    yp = yps[s % 2]
                    for k, zb in enumerate(act_banks):
                        hcol = (((sg * HQ) + (s % HQ)) * NZB + zb) * w
                        nc.tensor.matmul(
                            yp[:, :],
                            WL2[:, zb * 128:(zb + 1) * 128],
                            Hbuf[:, hcol:hcol + w],
                            start=(k == 0), stop=(k == len(act_banks) - 1),
                            skip_group_check=True,
                            tile_position=(0, 0))
                    sl = sg * XR_S + (s % XR_S)
                    nc.vector.tensor_scalar_add(
                        X[:, sl * w:(sl + 1) * w], yp[:], B2C[:])
                    if s - 5 >= 0:
                        dma_out(sg, s - 5)
            for so in range(max(0, nsw - 5), nsw):
                for sg in range(NSTREAM):
                    dma_out(sg, so)
    return nc


# ------------------------------------------------------------- host kernel
TRACE = False
LAST = None


def kernel(**inputs):
    noise = np.asarray(inputs["noise"], np.float32)      # [B, 64]
    W1 = np.asarray(inputs["W1"], np.float32)
    b1 = np.asarray(inputs["b1"], np.float32)
    W2 = np.asarray(inputs["W2"], np.float32)
    b2 = np.asarray(inputs["b2"], np.float32)
    # parent_idx is structurally fixed (banded DAG) — masking is baked into
    # the packed weights; int dtype preserved implicitly (unused on device).

    packed = pack_weights(W1, b1, W2, b2)
    build_bass._phase_nz = packed["phase_nz"]
    build_bass._segs = packed["segs"]

    nc = build_bass(w=W, c=C, l2cols=packed["l2cols"])
    nc.compile()

    import ml_dtypes
    bfnp = ml_dtypes.bfloat16
    ones = np.ones((4, NRING * W), bfnp)
    zeros = np.zeros((128, XRING * W), bfnp)
    noiseT = np.ascontiguousarray(noise.T)               # [64, B]
    in_maps = []
    for core in range(N_CORES):
        sh = np.ascontiguousarray(
            noiseT[:, core * B_SHARD:(core + 1) * B_SHARD]).astype(bfnp)
        in_maps.append(dict(noiseT=sh, wph=packed["wph"].astype(bfnp),
                            wl2=packed["wl2"].astype(bfnp),
                            b2c=packed["b2c"], ones=ones, zeros=zeros))

    from concourse.bass_utils import run_bass_kernel_spmd
    res = run_bass_kernel_spmd(nc, in_maps, core_ids=list(range(N_CORES)),
                               trace=TRACE)
    global LAST
    LAST = res
    gen = np.empty((noise.shape[0], NN), np.float32)
    for core in range(N_CORES):
        g = np.asarray(res.results[core]["gen"], np.float32)  # [64, B_SHARD]
        gen[core * B_SHARD:(core + 1) * B_SHARD, :] = g.T
    return gen



# revision 6
# speedup vs baseline: 1.2027x; 1.2027x over previous
"""Trainium2 Bass kernel for nn_CGNN_83605833384509.

Banded-DAG CGNN: gen[:, n] = MLP_n(gen[:, n-4:n] masked, noise[:, n]),
n = 0..63 sequential, B = 262144 batch.

Device strategy (unchanged from the working pipeline): data-parallel
over 8 cores (B/8 = 32768 each). Per core, a node-staggered software
pipeline ("superwaves"): at superwave s node n processes chunk c = s - n
(chunks of W=512 columns). Generated values live in a windowed,
partition-replicated SBUF ring tensor X so every matmul reads/writes
32-aligned partition windows. Per node: z = W1g.gen_parents + W1n.noise
+ b1 via accumulating 32x32-tile matmuls (3 nodes packed per matmul),
relu via ACT/DVE psum->SBUF evacuation, y = W2.h + b2 via embedded-
column matmuls, all active nodes' y written back to X in one 128-lane
op. Noise streams in / gen streams out via diagonal-in-DRAM DMA.

Host strategy (the part that dominates wall time — the axon tunnel runs
at ~60 MB/s shared both ways): ONE per-core DRAM input `nin`[80,32768]
bf16 packing the 64 noise rows plus the packed weights (16 rows), so
the whole upload is a single 40MB put to device 0 followed by a
terminal-side scatter (device-to-device, ~0.1s). SBUF state (X ring,
noise ring, bias-ones rows) is initialized with on-device memsets
instead of DMAing 32MB of literal zeros from the host. The donated
output buffers are recycled device arrays from the previous call (no
wire traffic). The built+compiled Bass module and the jitted sharded
executable are cached at module level, so repeat calls skip tracing and
compilation entirely. Output is fetched with 8 concurrent per-shard
reads (the tunnel serves parallel gets ~3x faster than one stream).
"""

import os
import time
import threading
import numpy as np

# ---------------------------------------------------------------- constants
NN = 64          # nodes
KP = 4           # max parents
NH = 10          # hidden width
W = 512          # chunk width (psum bank = 512 fp32)
C = 64           # chunks per core: B_shard = C*W = 32768
B_SHARD = C * W
N_CORES = 8
B_FULL = B_SHARD * N_CORES
NSTREAM = 4               # independent chunk-range streams (pipeline overlap)
CS = C // NSTREAM         # chunks per stream
NSW = CS + NN - 1         # superwaves per stream
XRING = 32                # gen ring slots total (16 per stream)
XR_S = XRING // NSTREAM
NRING = 16                # noise ring slots total (8 per stream)
NR_S = NRING // NSTREAM
NLAG = 2                  # noise refresh lead (superwaves), < NR_S
HQ = 2                    # Hbuf ring depth per stream
NZB = 6                   # z psum banks

# Packed single-input layout: rows 0..63 = noiseT, rows 64.. = weights
# (flat [128, WTS_COLS] region) followed by an 8192-long ones vector for
# the bias rows of XN (DMA can write single partitions; DVE memset
# cannot).
WTS_COLS = 0              # filled below once NTRIO known
NIN_WROWS = 17            # weight+ones rows appended after the noise rows
NIN_ROWS = NN + NIN_WROWS

# Windows: quadrant q holds gen rows for nodes [wlo, whi] at partition
# 32*q + (m - wlo).  Every trio's parents+self fit in its own window.
WIN = [(0, 14), (8, 29), (24, 45), (40, 63)]
NTRIO = 22
WTS_COLS = NTRIO * 5 * 30 + NZB * 128 + 1   # wph | wl2 | b2c
ONES_OFF = NN * C * W + 128 * WTS_COLS      # flat elem offset of ones vec
ONES_LEN = NRING * W


def trio_nodes(tau):
    return [n for n in range(3 * tau, min(3 * tau + 3, NN))]


def trio_win(tau):
    n0 = 3 * tau
    if n0 <= 12:
        return 0
    if n0 <= 27:
        return 1
    if n0 <= 42:
        return 2
    return 3


def win_rows(q):
    lo, hi = WIN[q]
    return hi - lo + 1


def pos_in_win(m, q):
    """partition row of gen-node m inside window q (must be present)."""
    lo, hi = WIN[q]
    assert lo <= m <= hi, (m, q)
    return 32 * q + (m - lo)


def windows_of(m):
    return [q for q in range(4) if WIN[q][0] <= m <= WIN[q][1]]


# z-psum placement: trio tau -> (zq, zb): quadrant zq = tau % 4, bank
# zb = tau // 4 (6 banks).  z rows = 32*zq .. 32*zq+29 (3 nodes x 10).
def trio_zq(tau):
    return tau % 4


def trio_zb(tau):
    return tau // 4


def active_range(s):
    return max(0, s - CS + 1), min(NN - 1, s)


def trio_active(tau, s):
    lo, hi = active_range(s)
    ns = trio_nodes(tau)
    return ns[0] <= hi and ns[-1] >= lo


# ------------------------------------------------------------- weight packing
def w1_row_for_parent(n, j):
    """W1 slot row holding the weight of parent m = n - j for node n."""
    if n >= KP:
        return KP - j
    return n - j  # left-aligned parents for n < 4


def pack_weights(W1, b1, W2, b2):
    """Build the merged packed weight array [128, WTS_COLS] (f32).

    Columns [0 : 3300)            wph  phase lhsT blocks
            [3300 : 3300+768)     wl2  L2 embedded lhsT segments
            [4068 : 4069)         b2c  y-evac bias
    Also returns phase_nz [NTRIO][5] (static given the banded DAG).
    """
    W1 = np.asarray(W1, np.float32)
    b1 = np.asarray(b1, np.float32)
    W2 = np.asarray(W2, np.float32)
    b2 = np.asarray(b2, np.float32)

    wts = np.zeros((128, WTS_COLS), np.float32)
    wph = wts[:, :NTRIO * 5 * 30]
    wl2 = wts[:, NTRIO * 5 * 30:NTRIO * 5 * 30 + NZB * 128]
    b2c = wts[:, WTS_COLS - 1:WTS_COLS]

    phase_nz = np.zeros((NTRIO, 5), bool)
    for tau in range(NTRIO):
        q = trio_win(tau)
        for j in range(5):
            off = (tau * 5 + j) * 30
            blk = wph[:, off:off + 30]
            for i, n in enumerate(trio_nodes(tau)):
                if j == 0:
                    # noise weights at node's own row + bias on ones-row 31
                    blk[pos_in_win(n, q), 10 * i:10 * i + 10] = W1[n, KP]
                    blk[32 * q + 31, 10 * i:10 * i + 10] = b1[n]
                    phase_nz[tau, j] = True
                else:
                    m = n - j
                    if m < 0:
                        continue
                    blk[pos_in_win(m, q), 10 * i:10 * i + 10] = \
                        W1[n, w1_row_for_parent(n, j)]
                    phase_nz[tau, j] = True

    # L2: one full-array (128 x 128) lhsT per z-bank: contracts the bank's
    # whole Hbuf column (its 4 trios), writes y at every window position of
    # its nodes (zero columns elsewhere); banks accumulate into y psum.
    for zb in range(NZB):
        blk = wl2[:, zb * 128:(zb + 1) * 128]
        for t in range(zb * 4, min(zb * 4 + 4, NTRIO)):
            zq = trio_zq(t)
            for i, n in enumerate(trio_nodes(t)):
                for oq in windows_of(n):
                    blk[32 * zq + 10 * i:32 * zq + 10 * i + 10,
                        pos_in_win(n, oq)] = W2[n]

    for m in range(NN):
        for q in windows_of(m):
            b2c[pos_in_win(m, q), 0] = b2[m]

    return wts, phase_nz


_PHASE_NZ = None  # filled by pack_weights caller before build_bass


# ------------------------------------------------------------- schedule
def xn_dma_jobs(sp):
    """Noise-refresh DMA jobs for superwave sp: list of
    (quad, row_a, nrows, n_lo, ring_slot, c_lo).  SBUF rows row_a.. get
    noise rows n_lo.. at chunk offsets c = sp - n (linear in n)."""
    lo, hi = active_range(sp)
    jobs = []
    by_q = {}
    for n in range(lo, hi + 1):
        q = trio_win(n // 3)
        by_q.setdefault(q, []).append(n)
    for q, ns in sorted(by_q.items()):
        n_lo, n_hi = ns[0], ns[-1]
        assert ns == list(range(n_lo, n_hi + 1))
        row_a = pos_in_win(n_lo, q)
        jobs.append((q, row_a, n_hi - n_lo + 1, n_lo, sp % NR_S, sp - n_lo))
    return jobs


def out_dma_jobs(sg):
    """Gen DMA-out jobs for slot written at superwave sg: list of
    (quad, row_a, nrows, m_lo, ring_slot, c_lo)."""
    lo, hi = active_range(sg)
    jobs = []
    bounds = [(0, 14), (15, 29), (30, 45), (46, 63)]
    for q, (plo, phi) in enumerate(bounds):
        m_lo, m_hi = max(lo, plo), min(hi, phi)
        if m_lo > m_hi:
            continue
        row_a = pos_in_win(m_lo, q)
        jobs.append((q, row_a, m_hi - m_lo + 1, m_lo, sg % XR_S, sg - m_lo))
    return jobs


# ------------------------------------------------------------- bass kernel
def build_bass(phase_nz, w=W, c=C):
    import concourse.bass as bass
    import concourse.bacc as bacc
    import concourse.mybir as mybir
    import concourse.tile as tile

    f32 = mybir.dt.float32
    bf16 = mybir.dt.bfloat16
    RELU = mybir.ActivationFunctionType.Relu

    nc = bacc.Bacc("TRN2", target_bir_lowering=False, debug=False,
                   enable_asserts=False, num_devices=N_CORES)

    # Single packed input: rows 0..63 noiseT, rows 64..79 the weights
    # (flat [128, WTS_COLS] region starting at row 64).
    d_nin = nc.dram_tensor("nin", [NIN_ROWS, c * w], bf16,
                           kind="ExternalInput").ap()
    d_gen = nc.dram_tensor("gen", [NN, c * w], bf16,
                           kind="ExternalOutput").ap()

    with tile.TileContext(nc) as tc:
        with tc.tile_pool(name="sb", bufs=1) as sb, \
             tc.tile_pool(name="ps", bufs=1, space="PSUM") as pp:
            cs = c // NSTREAM
            nsw = cs + NN - 1
            X = sb.tile([128, XRING * w], bf16)
            XN = sb.tile([128, NRING * w], bf16)
            Hbuf = sb.tile([128, NSTREAM * HQ * NZB * w], bf16)
            WTS = sb.tile([128, WTS_COLS], bf16)
            B2C = sb.tile([128, 1], f32)
            zpt = [pp.tile([128, 2 * w], f32, name=f"zpt{i}")
                   for i in range(NZB // 2)]
            yps = [pp.tile([128, w], f32, name=f"yps{i}") for i in range(2)]

            WPH_OFF = 0
            WL2_OFF = NTRIO * 5 * 30

            # weights: one contiguous DMA out of the packed input rows
            wsrc = bass.AP(d_nin.tensor, NN * c * w,
                           [[WTS_COLS, 128], [1, WTS_COLS]])
            nc.sync.dma_start(WTS[:], wsrc)
            # b2c column as f32 for the y-evacuation add
            nc.scalar.copy(B2C[:], WTS[:, WTS_COLS - 1:WTS_COLS])

            # SBUF state init on device (replaces 32MB of zeros DMA)
            nc.vector.memset(X[:], 0.0)
            nc.vector.memset(XN[:], 0.0)
            for qi in range(4):
                osrc = bass.AP(d_nin.tensor, ONES_OFF,
                               [[ONES_LEN, 1], [1, ONES_LEN]])
                nc.sync.dma_start(XN[32 * qi + 31:32 * qi + 32, :], osrc)
            for t in zpt:
                nc.vector.memset(t[:], 0.0)
            for t in yps:
                nc.vector.memset(t[:], 0.0)

            def xn_refresh(sg, sp):
                if sp >= nsw:
                    return
                cb = sg * cs
                for (q, row_a, nrows, n_lo, rs, c_lo) in xn_dma_jobs(sp):
                    k_ok = [k for k in range(nrows) if 0 <= c_lo - k < cs]
                    if not k_ok:
                        continue
                    k0, k1 = min(k_ok), max(k_ok)
                    off = (n_lo + k0) * c * w + (cb + c_lo - k0) * w
                    src_ap = bass.AP(d_nin.tensor, off,
                                     [[c * w - w, k1 - k0 + 1], [1, w]])
                    sl = sg * NR_S + rs
                    nc.sync.dma_start(
                        XN[row_a + k0:row_a + k1 + 1, sl * w:(sl + 1) * w],
                        src_ap)

            def dma_out(sg, so):
                cb = sg * cs
                for (q, row_a, nrows, m_lo, rs, c_lo) in out_dma_jobs(so):
                    off = m_lo * c * w + (cb + c_lo) * w
                    dst = bass.AP(d_gen.tensor, off,
                                  [[c * w - w, nrows], [1, w]])
                    sl = sg * XR_S + rs
                    nc.sync.dma_start(
                        dst, X[row_a:row_a + nrows, sl * w:(sl + 1) * w])

            for sg in range(NSTREAM):
                for sp in range(min(NLAG, nsw)):
                    xn_refresh(sg, sp)

            for t in range(nsw):
                for sg in range(NSTREAM):
                    s = t
                    xn_refresh(sg, s + NLAG)
                    act_trios = [tt for tt in range(NTRIO)
                                 if trio_active(tt, s)]
                    for tau in act_trios:
                        q, zq, zb = trio_win(tau), trio_zq(tau), trio_zb(tau)
                        js = [j for j in (0, 4, 3, 2, 1) if phase_nz[tau, j]]
                        for ji, j in enumerate(js):
                            off = WPH_OFF + (tau * 5 + j) * 30
                            if j == 0:
                                kw = 32
                                sl = sg * NR_S + (s % NR_S)
                                rhs = XN[32 * q:32 * q + 32,
                                         sl * w:(sl + 1) * w]
                            else:
                                kw = win_rows(q)
                                sl = sg * XR_S + ((s - j) % XR_S)
                                rhs = X[32 * q:32 * q + kw,
                                        sl * w:(sl + 1) * w]
                            lhsT = WTS[32 * q:32 * q + kw, off:off + 30]
                            nc.tensor.matmul(
                                zpt[zb // 2][32 * zq:32 * zq + 30,
                                             (zb % 2) * w:(zb % 2) * w + w],
                                lhsT, rhs,
                                start=(ji == 0), stop=(ji == len(js) - 1),
                                skip_group_check=True,
                                tile_position=(32 * q, 32 * zq))
                    act_banks0 = sorted({trio_zb(tt) for tt in act_trios})
                    act_pairs = sorted({zb // 2 for zb in act_banks0})
                    for bi, pb in enumerate(act_pairs):
                        hcol = (((sg * HQ) + (s % HQ)) * NZB + 2 * pb) * w
                        if bi % 2 == 0:
                            nc.scalar.activation(Hbuf[:, hcol:hcol + 2 * w],
                                                 zpt[pb][:], RELU)
                        else:
                            nc.vector.tensor_scalar_max(
                                Hbuf[:, hcol:hcol + 2 * w], zpt[pb][:], 0.0)
                    act_banks = [zb for pb in act_pairs
                                 for zb in (2 * pb, 2 * pb + 1)]
                    yp = yps[s % 2]
                    for k, zb in enumerate(act_banks):
                        hcol = (((sg * HQ) + (s % HQ)) * NZB + zb) * w
                        nc.tensor.matmul(
                            yp[:, :],
                            WTS[:, WL2_OFF + zb * 128:WL2_OFF + (zb + 1) * 128],
                            Hbuf[:, hcol:hcol + w],
                            start=(k == 0), stop=(k == len(act_banks) - 1),
                            skip_group_check=True,
                            tile_position=(0, 0))
                    sl = sg * XR_S + (s % XR_S)
                    nc.vector.tensor_scalar_add(
                        X[:, sl * w:(sl + 1) * w], yp[:], B2C[:])
                    if s - 5 >= 0:
                        dma_out(sg, s - 5)
            for so in range(max(0, nsw - 5), nsw):
                for sg in range(NSTREAM):
                    dma_out(sg, so)
    return nc


# ------------------------------------------------------------- host runtime
_RT = {}
_RT_LOCK = threading.Lock()
_DBG = os.environ.get("KT_DEBUG", "0") == "1"


def _dbg(msg, t0):
    if _DBG:
        import sys
        print(f"[kt] {msg}: {time.time() - t0:.3f}s", file=sys.stderr,
              flush=True)


def _ensure_state(phase_nz):
    st = _RT.get("st")
    if st is not None:
        return st
    import jax
    import jax.numpy as jnp
    from jax.sharding import Mesh, NamedSharding, PartitionSpec
    from jax.experimental.shard_map import shard_map
    import concourse.bass2jax as b2j
    import concourse.mybir as mybir

    t0 = time.time()
    nc = build_bass(phase_nz, w=W, c=C)
    _dbg("build_bass", t0)
    t0 = time.time()
    nc.compile()
    _dbg("nc.compile", t0)
    b2j.install_neuronx_cc_hook()
    assert nc.dbg_addr is None, "built with debug=False"

    devs = jax.devices()[:N_CORES]
    assert len(devs) == N_CORES
    mesh = Mesh(np.asarray(devs), ("core",))
    sh_core = NamedSharding(mesh, PartitionSpec("core"))

    partition_name = (nc.partition_id_tensor.name
                      if nc.partition_id_tensor else None)
    in_names, out_names, out_avals = [], [], []
    for alloc in nc.m.functions[0].allocations:
        if not isinstance(alloc, mybir.MemoryLocationSet):
            continue
        name = alloc.memorylocations[0].name
        if alloc.kind == "ExternalInput":
            if name != partition_name:
                in_names.append(name)
        elif alloc.kind == "ExternalOutput":
            out_names.append(name)
            out_avals.append(jax.core.ShapedArray(
                tuple(alloc.tensor_shape), mybir.dt.np(alloc.dtype)))
    assert in_names == ["nin"] and out_names == ["gen"], (in_names, out_names)
    all_names = in_names + out_names + (
        [partition_name] if partition_name else [])

    def _body(*args):
        operands = list(args)
        if partition_name:
            operands.append(b2j.partition_id_tensor())
        outs = b2j._bass_exec_p.bind(
            *operands,
            out_avals=tuple(out_avals),
            in_names=tuple(all_names),
            out_names=tuple(out_names),
            lowering_input_output_aliases=(),
            sim_require_finite=True,
            sim_require_nnan=True,
            nc=nc,
        )
        return tuple(outs)

    jitted = jax.jit(
        shard_map(_body, mesh=mesh,
                  in_specs=(PartitionSpec("core"), PartitionSpec("core")),
                  out_specs=(PartitionSpec("core"),), check_rep=False),
        donate_argnums=(1,), keep_unused=True)

    zeros_fn = jax.jit(
        lambda: jnp.zeros((N_CORES * NN, B_SHARD), jnp.bfloat16),
        out_shardings=sh_core)

    st = dict(nc=nc, jax=jax, devs=devs, mesh=mesh, sh_core=sh_core,
              jitted=jitted, zeros_fn=zeros_fn, donate_next=None)
    _RT["st"] = st
    return st


def kernel(**inputs):
    import ml_dtypes
    bfnp = ml_dtypes.bfloat16
    from concurrent.futures import ThreadPoolExecutor

    t_all = time.time()
    noise = np.asarray(inputs["noise"], np.float32)      # [B, 64]
    W1 = np.asarray(inputs["W1"], np.float32)
    b1 = np.asarray(inputs["b1"], np.float32)
    W2 = np.asarray(inputs["W2"], np.float32)
    b2 = np.asarray(inputs["b2"], np.float32)
    # parent_idx is structurally fixed (banded DAG) — masking is baked
    # into the packed weights; int dtype preserved implicitly (unused on
    # device).
    assert noise.shape == (B_FULL, NN), noise.shape

    t0 = time.time()
    wts, phase_nz = pack_weights(W1, b1, W2, b2)
    _dbg("pack_weights", t0)

    with _RT_LOCK:
        st = _ensure_state(phase_nz)
    jax = st["jax"]

    pool = _RT.get("pool")
    if pool is None:
        pool = ThreadPoolExecutor(max_workers=N_CORES)
        _RT["pool"] = pool

    # ---- build the packed host buffer [8, 80, 32768] bf16
    t0 = time.time()
    nbuf = _RT.get("nbuf")
    if nbuf is None:
        nbuf = np.empty((N_CORES, NIN_ROWS, B_SHARD), bfnp)
        _RT["nbuf"] = nbuf
    wrows = np.zeros((NIN_WROWS * B_SHARD,), bfnp)
    wrows[:128 * WTS_COLS] = wts.astype(bfnp).ravel()
    wrows[128 * WTS_COLS:128 * WTS_COLS + ONES_LEN] = 1.0
    wrows = wrows.reshape(NIN_WROWS, B_SHARD)
    n4 = noise.reshape(N_CORES, B_SHARD, NN)

    def _fill(i):
        nbuf[i, :NN, :] = n4[i].T      # f32 -> bf16 cast + transpose
        nbuf[i, NN:, :] = wrows
    list(pool.map(_fill, range(N_CORES)))
    _dbg("host pack buffer", t0)

    # ---- upload: one put to dev0, then terminal-side scatter
    t0 = time.time()
    g0 = jax.device_put(nbuf.reshape(N_CORES * NIN_ROWS, B_SHARD),
                        st["devs"][0])
    g0.block_until_ready()
    _dbg("put 40MB", t0)
    t0 = time.time()
    nin_g = jax.device_put(g0, st["sh_core"])
    nin_g.block_until_ready()
    del g0
    _dbg("scatter", t0)

    # ---- donated output buffer: recycle last call's (or fresh zeros)
    don = st["donate_next"]
    st["donate_next"] = None
    if don is None:
        don = st["zeros_fn"]()

    # ---- execute
    t0 = time.time()
    (out_g,) = st["jitted"](nin_g, don)
    out_g.block_until_ready()
    _dbg("exec", t0)

    # ---- fetch: 8 concurrent per-shard gets + transpose into output
    t0 = time.time()
    shards = sorted(out_g.addressable_shards,
                    key=lambda s: s.index[0].start or 0)
    assert len(shards) == N_CORES
    gen = np.empty((B_FULL, NN), np.float32)

    def _fetch(i):
        a = np.asarray(shards[i].data)           # [64, 32768] bf16
        gen[i * B_SHARD:(i + 1) * B_SHARD, :] = a.T
    list(pool.map(_fetch, range(N_CORES)))
    _dbg("fetch+gather", t0)

    st["donate_next"] = out_g
    _dbg("kernel total", t_all)
    return gen
